# revision 1
# baseline (speedup 1.0000x reference)
"""BertAttention (with additive KV injection) Trainium2 kernel.

Problem: nn_BertAttention_12781822673413
  B=4, S=2048, DM=768, H=12 heads, HD=64, NSYN=4 (additive k/v on first 4 heads)
  out = LayerNorm(attn_out @ Wo.T + bo + x) * ln_g + ln_b

Sharding: 8 cores = (batch b, query-half) pairs.  Each core computes q for its
1024-token half, k/v for the full 2048 sequence of its batch (k/v projection is
duplicated across the 2 cores of a batch - this avoids any collective), runs
12 heads of attention for its query half, output projection, residual + LN.
No collectives; outputs are disjoint slices of the full output.

Device layouts (per core):
  xT    [768,2048] bf16  - x[b].T with key order [own_half, other_half]
  qT    [768,1024] bf16  - q.T  (head h = partitions h*64..h*64+64)
  kT    [768,2048] bf16  - k.T (+ additive key on heads 0-3)
  v_aug [2048,780] bf16  - v in [t,(h,d)] with a ones column per head (65 wide)
                           -> PV matmul row 64 yields the softmax denominator
  scores_T [t,s] psum; exp fused on ScalarE (scale=1/8, bias=mask[t]); PV gives
  ctx.T [65,1024] psum; normalize by denominator; out proj from ctx.T as lhsT.

The zero-valued biases (bq,bk,bv) get dedicated instructions only when nonzero
(decided at trace time from the actual input values); bo is folded into the
residual input on the host; ln_g/ln_b are applied on the host when nontrivial.
"""

import os
import sys

for _p in ("/opt/trn_rl_repo", "/root/.axon_site/_ro/trn_rl_repo"):
    if os.path.isdir(_p) and _p not in sys.path:
        sys.path.insert(0, _p)

from contextlib import ExitStack

import ml_dtypes
import numpy as np

import concourse.bass as bass
import concourse.tile as tile
from concourse import bacc, mybir
from concourse.bass_utils import run_bass_kernel_spmd

BF16 = ml_dtypes.bfloat16

B, S, DM, H, NSYN = 4, 2048, 768, 12, 4
HD = DM // H            # 64
SH = S // 2             # 1024 queries per core
P = 128
NT = S // P             # 16 key tiles
NJ = DM // P            # 6 model-dim tiles
NSH = SH // P           # 8 query tiles
SCALE = float(DM / H) ** -0.5   # 0.125
EPS = 1e-12
N_CORES = 8

f32 = mybir.dt.float32
bf16 = mybir.dt.bfloat16

AF = mybir.ActivationFunctionType
ALU = mybir.AluOpType


def _build_program(bq_nz: bool, bk_nz: bool, bv_nz: bool):
    nc = bacc.Bacc(
        "TRN2",
        target_bir_lowering=False,
        debug=False,
        enable_asserts=False,
        num_devices=N_CORES,
    )

    xT = nc.dram_tensor("xT", [DM, S], bf16, kind="ExternalInput").ap()
    xr = nc.dram_tensor("xr", [SH, DM], f32, kind="ExternalInput").ap()
    wq = nc.dram_tensor("wqT", [DM, DM], bf16, kind="ExternalInput").ap()
    wk = nc.dram_tensor("wkT", [DM, DM], bf16, kind="ExternalInput").ap()
    wv = nc.dram_tensor("wvT", [DM, DM], bf16, kind="ExternalInput").ap()
    wo = nc.dram_tensor("woT", [DM, DM], bf16, kind="ExternalInput").ap()
    addikT = nc.dram_tensor("addikT", [NSYN * HD, S], bf16, kind="ExternalInput").ap()
    addiv = nc.dram_tensor("addiv", [S, NSYN * HD], bf16, kind="ExternalInput").ap()
    maskd = nc.dram_tensor("mask", [S], f32, kind="ExternalInput").ap()
    bqd = nc.dram_tensor("bq", [DM], f32, kind="ExternalInput").ap()
    bkd = nc.dram_tensor("bk", [DM], f32, kind="ExternalInput").ap()
    bvd = nc.dram_tensor("bv", [DM], f32, kind="ExternalInput").ap()
    out = nc.dram_tensor("out", [SH, DM], f32, kind="ExternalOutput").ap()

    with tile.TileContext(nc) as tc, ExitStack() as ctx:
        const = ctx.enter_context(tc.tile_pool(name="const", bufs=1))

        xT_sb = const.tile([P, NJ, S], bf16, name="xT_sb")
        wq_sb = const.tile([P, NJ, DM], bf16, name="wq_sb")
        wk_sb = const.tile([P, NJ, DM], bf16, name="wk_sb")
        wv_sb = const.tile([P, NJ, DM], bf16, name="wv_sb")
        wo_sb = const.tile([P, NJ, DM], bf16, name="wo_sb")
        qT_sb = const.tile([P, NJ, SH], bf16, name="qT_sb")
        kT_sb = const.tile([P, NJ, S], bf16, name="kT_sb")
        vaug_sb = const.tile([P, NT, H * (HD + 1)], bf16, name="vaug_sb")
        ctxT_sb = const.tile([P, NJ, SH], bf16, name="ctxT_sb")
        mask_sb = const.tile([P, NT], f32, name="mask_sb")
        ones_sb = const.tile([1, HD], f32, name="ones_sb")
        eps_sb = const.tile([P, 1], f32, name="eps_sb")

        for it in range(NJ):
            nc.sync.dma_start(xT_sb[:, it, :], xT[it * P : (it + 1) * P, :])
        for w_sb, w_dram in ((wq_sb, wq), (wk_sb, wk), (wv_sb, wv), (wo_sb, wo)):
            for it in range(NJ):
                nc.sync.dma_start(w_sb[:, it, :], w_dram[it * P : (it + 1) * P, :])
        nc.sync.dma_start(mask_sb[:], maskd.rearrange("(t p) -> p t", p=P))
        nc.vector.memset(ones_sb[:], 1.0)
        nc.vector.memset(eps_sb[:], EPS)
        # ones columns of v_aug (offset 64 of every 65-wide head block) survive
        # the projection writes below, which only cover offsets 0..63.
        nc.gpsimd.memset(vaug_sb[:], 1.0)

        bias_tiles = {}
        for nz, nm, dram in ((bq_nz, "bq", bqd), (bk_nz, "bk", bkd), (bv_nz, "bv", bvd)):
            if nz:
                t = const.tile([P, NJ], f32, name=f"{nm}_sb")
                nc.sync.dma_start(t[:], dram.rearrange("(t p) -> p t", p=P))
                bias_tiles[nm] = t

        ps = ctx.enter_context(tc.tile_pool(name="ps", bufs=2, space="PSUM"))
        psc = ctx.enter_context(tc.tile_pool(name="psc", bufs=2, space="PSUM"))
        ppool = ctx.enter_context(tc.tile_pool(name="ppool", bufs=3))
        akpool = ctx.enter_context(tc.tile_pool(name="akpool", bufs=2))
        avpool = ctx.enter_context(tc.tile_pool(name="avpool", bufs=2))
        bcpool = ctx.enter_context(tc.tile_pool(name="bcpool", bufs=2))
        rcpool = ctx.enter_context(tc.tile_pool(name="rcpool", bufs=2))
        xrpool = ctx.enter_context(tc.tile_pool(name="xrpool", bufs=2))
        hpool = ctx.enter_context(tc.tile_pool(name="hpool", bufs=2))
        opool = ctx.enter_context(tc.tile_pool(name="opool", bufs=2))
        stpool = ctx.enter_context(tc.tile_pool(name="stpool", bufs=3))

        def psum_tile(name):
            return ps.tile([P, 1024], f32, name=name, tag="ps")

        # ---- Phase 1a: qT[j, s] = sum_i WqT[i, j] * xT[i, s(own half)] ----
        for jt in range(NJ):
            psq = psum_tile(f"psq{jt}")
            for it in range(NJ):
                lhs = wq_sb[:, it, jt * P : (jt + 1) * P]
                for c0 in (0, 512):
                    nc.tensor.matmul(
                        psq[:, c0 : c0 + 512],
                        lhsT=lhs,
                        rhs=xT_sb[:, it, c0 : c0 + 512],
                        start=(it == 0),
                        stop=(it == NJ - 1),
                    )
            dest = qT_sb[:, jt, :]
            if bq_nz:
                nc.scalar.activation(
                    dest, psq[:], AF.Identity, bias=bias_tiles["bq"][:, jt : jt + 1]
                )
            else:
                nc.any.tensor_copy(out=dest, in_=psq[:])

        # ---- Phase 1b: kT[j, t] (+ additive key on heads 0..3) ----
        for jt in range(NJ):
            for th in range(2):
                psk = psum_tile(f"psk{jt}_{th}")
                for it in range(NJ):
                    lhs = wk_sb[:, it, jt * P : (jt + 1) * P]
                    for c0 in (0, 512):
                        nc.tensor.matmul(
                            psk[:, c0 : c0 + 512],
                            lhsT=lhs,
                            rhs=xT_sb[:, it, th * 1024 + c0 : th * 1024 + c0 + 512],
                            start=(it == 0),
                            stop=(it == NJ - 1),
                        )
                dest = kT_sb[:, jt, th * 1024 : (th + 1) * 1024]
                if jt < 2:  # heads 0..3 live on partition tiles 0 and 1
                    ak = akpool.tile([P, 1024], bf16, name="ak", tag="ak")
                    nc.sync.dma_start(
                        ak[:],
                        addikT[jt * P : (jt + 1) * P, th * 1024 : (th + 1) * 1024],
                    )
                    nc.vector.tensor_add(out=dest, in0=psk[:], in1=ak[:])
                    if bk_nz:
                        nc.vector.tensor_scalar_add(
                            dest, dest, bias_tiles["bk"][:, jt : jt + 1]
                        )
                else:
                    if bk_nz:
                        nc.scalar.activation(
                            dest, psk[:], AF.Identity,
                            bias=bias_tiles["bk"][:, jt : jt + 1],
                        )
                    else:
                        nc.any.tensor_copy(out=dest, in_=psk[:])

        # ---- Phase 1c: v[t, j] into v_aug (+ additive value on heads 0..3) ----
        for tt in range(NT):
            psv = psum_tile(f"psv{tt}")
            for it in range(NJ):
                lhs = xT_sb[:, it, tt * P : (tt + 1) * P]
                nc.tensor.matmul(
                    psv[:, 0:512], lhsT=lhs, rhs=wv_sb[:, it, 0:512],
                    start=(it == 0), stop=(it == NJ - 1),
                )
                nc.tensor.matmul(
                    psv[:, 512:768], lhsT=lhs, rhs=wv_sb[:, it, 512:768],
                    start=(it == 0), stop=(it == NJ - 1),
                )
            vrow = vaug_sb[:, tt, :].rearrange("p (h e) -> p h e", e=HD + 1)
            av = avpool.tile([P, NSYN * HD], bf16, name="av", tag="av")
            nc.sync.dma_start(av[:], addiv[tt * P : (tt + 1) * P, :])
            nc.vector.tensor_add(
                out=vrow[:, 0:NSYN, 0:HD],
                in0=psv[:, 0 : NSYN * HD].rearrange("p (h e) -> p h e", e=HD),
                in1=av[:].rearrange("p (h e) -> p h e", e=HD),
            )
            nc.any.tensor_copy(
                out=vrow[:, NSYN:H, 0:HD],
                in_=psv[:, NSYN * HD : DM].rearrange("p (h e) -> p h e", e=HD),
            )

        # ---- Phase 2: attention per head ----
        for h in range(H):
            it = h // 2
            po = (h % 2) * HD
            kTh = kT_sb[po : po + HD, it, :]
            qTh = qT_sb[po : po + HD, it, :]
            psctx = psc.tile([HD + 1, 1024], f32, name=f"ctx{h}", tag="ctx")
            for tt in range(NT):
                pss = psum_tile(f"pss{h}_{tt}")
                for c0 in (0, 512):
                    nc.tensor.matmul(
                        pss[:, c0 : c0 + 512],
                        lhsT=kTh[:, tt * P : (tt + 1) * P],
                        rhs=qTh[:, c0 : c0 + 512],
                        start=True,
                        stop=True,
                    )
                pt = ppool.tile([P, 1024], bf16, name="pt", tag="pt")
                nc.scalar.activation(
                    pt[:], pss[:], AF.Exp,
                    bias=mask_sb[:, tt : tt + 1], scale=SCALE,
                )
                for c0 in (0, 512):
                    nc.tensor.matmul(
                        psctx[:, c0 : c0 + 512],
                        lhsT=vaug_sb[:, tt, h * (HD + 1) : (h + 1) * (HD + 1)],
                        rhs=pt[:, c0 : c0 + 512],
                        start=(tt == 0),
                        stop=(tt == NT - 1),
                    )
            rc = rcpool.tile([1, 1024], f32, name="rc", tag="rc")
            nc.vector.reciprocal(rc[:], psctx[HD : HD + 1, :])
            psb = psum_tile(f"psb{h}")
            for c0 in (0, 512):
                nc.tensor.matmul(
                    psb[0:HD, c0 : c0 + 512],
                    lhsT=ones_sb[:],
                    rhs=rc[:, c0 : c0 + 512],
                    start=True,
                    stop=True,
                )
            bc = bcpool.tile([HD, 1024], f32, name="bc", tag="bc")
            nc.any.tensor_copy(out=bc[:], in_=psb[0:HD, :])
            dest = ctxT_sb[po : po + HD, it, :]
            nc.vector.tensor_mul(out=dest, in0=psctx[0:HD, :], in1=bc[:])
            if bv_nz:
                nc.vector.tensor_scalar_add(
                    dest, dest, bias_tiles["bv"][po : po + HD, it : it + 1]
                )

        # ---- Phase 3: out proj + residual + LayerNorm ----
        for sc in range(NSH):
            pso = psum_tile(f"pso{sc}")
            for it in range(NJ):
                lhs = ctxT_sb[:, it, sc * P : (sc + 1) * P]
                nc.tensor.matmul(
                    pso[:, 0:512], lhsT=lhs, rhs=wo_sb[:, it, 0:512],
                    start=(it == 0), stop=(it == NJ - 1),
                )
                nc.tensor.matmul(
                    pso[:, 512:768], lhsT=lhs, rhs=wo_sb[:, it, 512:768],
                    start=(it == 0), stop=(it == NJ - 1),
                )
            xrt = xrpool.tile([P, DM], f32, name="xrt", tag="xr")
            nc.sync.dma_start(xrt[:], xr[sc * P : (sc + 1) * P, :])
            ht = hpool.tile([P, DM], f32, name="ht", tag="h")
            nc.vector.tensor_add(out=ht[:], in0=pso[:, 0:DM], in1=xrt[:])
            stats = stpool.tile([P, 3, 6], f32, name="stats", tag="st")
            for g in range(3):
                nc.vector.bn_stats(stats[:, g, :], ht[:, g * 256 : (g + 1) * 256])
            mv = stpool.tile([P, 2], f32, name="mv", tag="mv")
            nc.vector.bn_aggr(mv[:], stats[:])
            sq = stpool.tile([P, 1], f32, name="sq", tag="sq")
            nc.scalar.activation(sq[:], mv[:, 1:2], AF.Sqrt, bias=eps_sb[:])
            rstd = stpool.tile([P, 1], f32, name="rstd", tag="rstd")
            nc.vector.reciprocal(rstd[:], sq[:])
            ot = opool.tile([P, DM], f32, name="ot", tag="ot")
            nc.vector.tensor_scalar(
                out=ot[:], in0=ht[:],
                scalar1=mv[:, 0:1], scalar2=rstd[:],
                op0=ALU.subtract, op1=ALU.mult,
            )
            nc.sync.dma_start(out[sc * P : (sc + 1) * P, :], ot[:])

    nc.compile()
    return nc


_PROGRAM_CACHE: dict = {}


def _get_program(bq_nz, bk_nz, bv_nz):
    key = (bq_nz, bk_nz, bv_nz)
    if key not in _PROGRAM_CACHE:
        _PROGRAM_CACHE[key] = _build_program(*key)
    return _PROGRAM_CACHE[key]


def _prep_core_inputs(inputs, b, half):
    """Host-side shard prep for core (b, half). Keys are permuted so the core's
    own query half comes first; attention is permutation-invariant in t as long
    as k, v, mask and the additive tensors share the order."""
    x = np.asarray(inputs["hidden_states"][b], np.float32)          # [S, DM]
    if half == 0:
        t_order = slice(None)
        xh = x[:SH]
    else:
        t_order = np.r_[SH:S, 0:SH]
        xh = x[SH:]
    xp = x[t_order] if half else x                                  # [S, DM] permuted
    xT = np.ascontiguousarray(xp.T).astype(BF16)                    # [DM, S]
    xr = xh + np.asarray(inputs["bo"], np.float32)[None, :]         # residual + bo
    ak = np.asarray(inputs["addi_key"][b], np.float32)              # [NSYN, S, HD]
    ak = ak.transpose(0, 2, 1).reshape(NSYN * HD, S)
    av = np.asarray(inputs["addi_value"][b], np.float32)
    av = av.transpose(1, 0, 2).reshape(S, NSYN * HD)
    mask = np.asarray(inputs["attention_mask"][b, 0, 0], np.float32)
    if half:
        ak = ak[:, t_order]
        av = av[t_order]
        mask = mask[t_order]
    return {
        "xT": xT,
        "xr": np.ascontiguousarray(xr, np.float32),
        "addikT": np.ascontiguousarray(ak).astype(BF16),
        "addiv": np.ascontiguousarray(av).astype(BF16),
        "mask": np.ascontiguousarray(mask, np.float32),
    }


def _prep_in_maps(inputs):
    wqT = np.ascontiguousarray(np.asarray(inputs["Wq"], np.float32).T).astype(BF16)
    wkT = np.ascontiguousarray(np.asarray(inputs["Wk"], np.float32).T).astype(BF16)
    wvT = np.ascontiguousarray(np.asarray(inputs["Wv"], np.float32).T).astype(BF16)
    woT = np.ascontiguousarray(np.asarray(inputs["Wo"], np.float32).T).astype(BF16)
    shared = {
        "wqT": wqT, "wkT": wkT, "wvT": wvT, "woT": woT,
        "bq": np.asarray(inputs["bq"], np.float32),
        "bk": np.asarray(inputs["bk"], np.float32),
        "bv": np.asarray(inputs["bv"], np.float32),
    }
    in_maps = []
    for c in range(N_CORES):
        m = _prep_core_inputs(inputs, c // 2, c % 2)
        m.update(shared)
        in_maps.append(m)
    return in_maps


def _postprocess(inputs, results):
    out = np.empty((B, S, DM), np.float32)
    for c in range(N_CORES):
        b, half = c // 2, c % 2
        out[b, half * SH : (half + 1) * SH] = results[c]["out"]
    ln_g = np.asarray(inputs["ln_g"], np.float32)
    ln_b = np.asarray(inputs["ln_b"], np.float32)
    if np.any(ln_b) or not np.all(ln_g == 1.0):
        out = out * ln_g[None, None, :] + ln_b[None, None, :]
    return out


def run(inputs, trace=False, **kwargs):
    """Run on hardware; returns (full_output, BassKernelResults)."""
    nc = _get_program(
        bool(np.any(inputs["bq"])),
        bool(np.any(inputs["bk"])),
        bool(np.any(inputs["bv"])),
    )
    in_maps = _prep_in_maps(inputs)
    res = run_bass_kernel_spmd(
        nc, in_maps, core_ids=list(range(N_CORES)), trace=trace, **kwargs
    )
    return _postprocess(inputs, res.results), res


def kernel(**inputs) -> np.ndarray:
    out, _ = run(inputs)
    return out


# revision 2
# speedup vs baseline: 1.2210x; 1.2210x over previous
"""BertAttention (with additive KV injection) Trainium2 kernel.

Problem: nn_BertAttention_12781822673413
  B=4, S=2048, DM=768, H=12 heads, HD=64, NSYN=4 (additive k/v on first 4 heads)
  out = LayerNorm(attn_out @ Wo.T + bo + x) * ln_g + ln_b

Sharding: 8 cores = (batch b, query-half) pairs.  Each core computes q for its
1024-token half, k/v for the full 2048 sequence of its batch (k/v projection is
duplicated across the 2 cores of a batch - this avoids any collective), runs
12 heads of attention for its query half, output projection, residual + LN.
No collectives; outputs are disjoint slices of the full output.

Device layouts (per core):
  xT    [768,2048] bf16  - x[b].T with key order [own_half, other_half]
  qT    [768,1024] bf16  - q.T  (head h = partitions h*64..h*64+64)
  kT    [768,2048] bf16  - k.T (+ additive key on heads 0-3)
  v_aug [2048,780] bf16  - v in [t,(h,d)] with a ones column per head (65 wide)
                           -> PV matmul row 64 yields the softmax denominator
  scores_T [t,s] psum; exp fused on ScalarE (scale=1/8, bias=mask[t]); PV gives
  ctx.T [65,1024] psum; normalize by denominator; out proj from ctx.T as lhsT.

The zero-valued biases (bq,bk,bv) get dedicated instructions only when nonzero
(decided at trace time from the actual input values); bo is folded into the
residual input on the host; ln_g/ln_b are applied on the host when nontrivial.
"""

import os
import sys

for _p in ("/opt/trn_rl_repo", "/root/.axon_site/_ro/trn_rl_repo"):
    if os.path.isdir(_p) and _p not in sys.path:
        sys.path.insert(0, _p)

from contextlib import ExitStack

import ml_dtypes
import numpy as np

import concourse.bass as bass
import concourse.tile as tile
from concourse import bacc, mybir
from concourse.bass_utils import run_bass_kernel_spmd

BF16 = ml_dtypes.bfloat16

B, S, DM, H, NSYN = 4, 2048, 768, 12, 4
HD = DM // H            # 64
SH = S // 2             # 1024 queries per core
P = 128
NT = S // P             # 16 key tiles
NJ = DM // P            # 6 model-dim tiles
NSH = SH // P           # 8 query tiles
SCALE = float(DM / H) ** -0.5   # 0.125
EPS = 1e-12
N_CORES = 8

f32 = mybir.dt.float32
bf16 = mybir.dt.bfloat16

AF = mybir.ActivationFunctionType
ALU = mybir.AluOpType


def _build_program(bq_nz: bool, bk_nz: bool, bv_nz: bool):
    nc = bacc.Bacc(
        "TRN2",
        target_bir_lowering=False,
        debug=False,
        enable_asserts=False,
        num_devices=N_CORES,
    )

    xT = nc.dram_tensor("xT", [DM, S], bf16, kind="ExternalInput").ap()
    xr = nc.dram_tensor("xr", [SH, DM], f32, kind="ExternalInput").ap()
    wq = nc.dram_tensor("wqT", [DM, DM], bf16, kind="ExternalInput").ap()
    wk = nc.dram_tensor("wkT", [DM, DM], bf16, kind="ExternalInput").ap()
    wv = nc.dram_tensor("wvT", [DM, DM], bf16, kind="ExternalInput").ap()
    wo = nc.dram_tensor("woT", [DM, DM], bf16, kind="ExternalInput").ap()
    addikT = nc.dram_tensor("addikT", [NSYN * HD, S], bf16, kind="ExternalInput").ap()
    addiv = nc.dram_tensor("addiv", [S, NSYN * HD], bf16, kind="ExternalInput").ap()
    maskd = nc.dram_tensor("mask", [S], f32, kind="ExternalInput").ap()
    bqd = nc.dram_tensor("bq", [DM], f32, kind="ExternalInput").ap()
    bkd = nc.dram_tensor("bk", [DM], f32, kind="ExternalInput").ap()
    bvd = nc.dram_tensor("bv", [DM], f32, kind="ExternalInput").ap()
    out = nc.dram_tensor("out", [SH, DM], f32, kind="ExternalOutput").ap()

    with tile.TileContext(nc) as tc, ExitStack() as ctx:
        const = ctx.enter_context(tc.tile_pool(name="const", bufs=1))

        xT_sb = const.tile([P, NJ, S], bf16, name="xT_sb")
        wq_sb = const.tile([P, NJ, DM], bf16, name="wq_sb")
        wk_sb = const.tile([P, NJ, DM], bf16, name="wk_sb")
        wv_sb = const.tile([P, NJ, DM], bf16, name="wv_sb")
        wo_sb = const.tile([P, NJ, DM], bf16, name="wo_sb")
        qT_sb = const.tile([P, NJ, SH], bf16, name="qT_sb")
        kT_sb = const.tile([P, NJ, S], bf16, name="kT_sb")
        vaug_sb = const.tile([P, NT, H * (HD + 1)], bf16, name="vaug_sb")
        ctxT_sb = const.tile([P, NJ, SH], bf16, name="ctxT_sb")
        mask_sb = const.tile([P, NT], f32, name="mask_sb")
        ones_sb = const.tile([1, HD], f32, name="ones_sb")
        eps_sb = const.tile([P, 1], f32, name="eps_sb")

        for it in range(NJ):
            nc.sync.dma_start(xT_sb[:, it, :], xT[it * P : (it + 1) * P, :])
        for w_sb, w_dram in ((wq_sb, wq), (wk_sb, wk), (wv_sb, wv), (wo_sb, wo)):
            for it in range(NJ):
                nc.sync.dma_start(w_sb[:, it, :], w_dram[it * P : (it + 1) * P, :])
        nc.sync.dma_start(mask_sb[:], maskd.rearrange("(t p) -> p t", p=P))
        nc.vector.memset(ones_sb[:], 1.0)
        nc.vector.memset(eps_sb[:], EPS)
        # ones columns of v_aug (offset 64 of every 65-wide head block) survive
        # the projection writes below, which only cover offsets 0..63.
        nc.gpsimd.memset(vaug_sb[:], 1.0)

        bias_tiles = {}
        for nz, nm, dram in ((bq_nz, "bq", bqd), (bk_nz, "bk", bkd), (bv_nz, "bv", bvd)):
            if nz:
                t = const.tile([P, NJ], f32, name=f"{nm}_sb")
                nc.sync.dma_start(t[:], dram.rearrange("(t p) -> p t", p=P))
                bias_tiles[nm] = t

        ps = ctx.enter_context(tc.tile_pool(name="ps", bufs=2, space="PSUM"))
        psc = ctx.enter_context(tc.tile_pool(name="psc", bufs=2, space="PSUM"))
        ppool = ctx.enter_context(tc.tile_pool(name="ppool", bufs=3))
        akpool = ctx.enter_context(tc.tile_pool(name="akpool", bufs=2))
        avpool = ctx.enter_context(tc.tile_pool(name="avpool", bufs=2))
        bcpool = ctx.enter_context(tc.tile_pool(name="bcpool", bufs=2))
        rcpool = ctx.enter_context(tc.tile_pool(name="rcpool", bufs=2))
        xrpool = ctx.enter_context(tc.tile_pool(name="xrpool", bufs=2))
        hpool = ctx.enter_context(tc.tile_pool(name="hpool", bufs=2))
        opool = ctx.enter_context(tc.tile_pool(name="opool", bufs=2))
        stpool = ctx.enter_context(tc.tile_pool(name="stpool", bufs=3))

        def psum_tile(name):
            return ps.tile([P, 1024], f32, name=name, tag="ps")

        # ---- Phase 1a: qT[j, s] = sum_i WqT[i, j] * xT[i, s(own half)] ----
        for jt in range(NJ):
            psq = psum_tile(f"psq{jt}")
            for it in range(NJ):
                lhs = wq_sb[:, it, jt * P : (jt + 1) * P]
                for c0 in (0, 512):
                    nc.tensor.matmul(
                        psq[:, c0 : c0 + 512],
                        lhsT=lhs,
                        rhs=xT_sb[:, it, c0 : c0 + 512],
                        start=(it == 0),
                        stop=(it == NJ - 1),
                    )
            dest = qT_sb[:, jt, :]
            if bq_nz:
                nc.scalar.activation(
                    dest, psq[:], AF.Identity, bias=bias_tiles["bq"][:, jt : jt + 1]
                )
            else:
                nc.any.tensor_copy(out=dest, in_=psq[:])

        # ---- Phase 1b: kT[j, t] (+ additive key on heads 0..3) ----
        for jt in range(NJ):
            for th in range(2):
                psk = psum_tile(f"psk{jt}_{th}")
                for it in range(NJ):
                    lhs = wk_sb[:, it, jt * P : (jt + 1) * P]
                    for c0 in (0, 512):
                        nc.tensor.matmul(
                            psk[:, c0 : c0 + 512],
                            lhsT=lhs,
                            rhs=xT_sb[:, it, th * 1024 + c0 : th * 1024 + c0 + 512],
                            start=(it == 0),
                            stop=(it == NJ - 1),
                        )
                dest = kT_sb[:, jt, th * 1024 : (th + 1) * 1024]
                if jt < 2:  # heads 0..3 live on partition tiles 0 and 1
                    ak = akpool.tile([P, 1024], bf16, name="ak", tag="ak")
                    nc.sync.dma_start(
                        ak[:],
                        addikT[jt * P : (jt + 1) * P, th * 1024 : (th + 1) * 1024],
                    )
                    nc.vector.tensor_add(out=dest, in0=psk[:], in1=ak[:])
                    if bk_nz:
                        nc.vector.tensor_scalar_add(
                            dest, dest, bias_tiles["bk"][:, jt : jt + 1]
                        )
                else:
                    if bk_nz:
                        nc.scalar.activation(
                            dest, psk[:], AF.Identity,
                            bias=bias_tiles["bk"][:, jt : jt + 1],
                        )
                    else:
                        nc.any.tensor_copy(out=dest, in_=psk[:])

        # ---- Phase 1c: v[t, j] into v_aug (+ additive value on heads 0..3) ----
        for tt in range(NT):
            psv = psum_tile(f"psv{tt}")
            for it in range(NJ):
                lhs = xT_sb[:, it, tt * P : (tt + 1) * P]
                nc.tensor.matmul(
                    psv[:, 0:512], lhsT=lhs, rhs=wv_sb[:, it, 0:512],
                    start=(it == 0), stop=(it == NJ - 1),
                )
                nc.tensor.matmul(
                    psv[:, 512:768], lhsT=lhs, rhs=wv_sb[:, it, 512:768],
                    start=(it == 0), stop=(it == NJ - 1),
                )
            vrow = vaug_sb[:, tt, :].rearrange("p (h e) -> p h e", e=HD + 1)
            av = avpool.tile([P, NSYN * HD], bf16, name="av", tag="av")
            nc.sync.dma_start(av[:], addiv[tt * P : (tt + 1) * P, :])
            nc.vector.tensor_add(
                out=vrow[:, 0:NSYN, 0:HD],
                in0=psv[:, 0 : NSYN * HD].rearrange("p (h e) -> p h e", e=HD),
                in1=av[:].rearrange("p (h e) -> p h e", e=HD),
            )
            nc.any.tensor_copy(
                out=vrow[:, NSYN:H, 0:HD],
                in_=psv[:, NSYN * HD : DM].rearrange("p (h e) -> p h e", e=HD),
            )

        # ---- Phase 2: attention per head ----
        # The normalize of head h is emitted after head h+1's score/PV loop so
        # the (in-order) PE queue never stalls waiting on the DVE reciprocal:
        # by the time PE reaches head h's broadcast matmul, the reciprocal has
        # long finished behind head h+1's dense matmul work.
        ctx_tiles = {}

        def normalize(h):
            it = h // 2
            po = (h % 2) * HD
            psctx = ctx_tiles.pop(h)
            rc = rcpool.tile([1, 1024], f32, name="rc", tag="rc")
            nc.vector.reciprocal(rc[:], psctx[HD : HD + 1, :])
            psb = psum_tile(f"psb{h}")
            for c0 in (0, 512):
                nc.tensor.matmul(
                    psb[0:HD, c0 : c0 + 512],
                    lhsT=ones_sb[:],
                    rhs=rc[:, c0 : c0 + 512],
                    start=True,
                    stop=True,
                )
            bc = bcpool.tile([HD, 1024], f32, name="bc", tag="bc")
            nc.any.tensor_copy(out=bc[:], in_=psb[0:HD, :])
            dest = ctxT_sb[po : po + HD, it, :]
            nc.vector.tensor_mul(out=dest, in0=psctx[0:HD, :], in1=bc[:])
            if bv_nz:
                nc.vector.tensor_scalar_add(
                    dest, dest, bias_tiles["bv"][po : po + HD, it : it + 1]
                )

        for h in range(H):
            it = h // 2
            po = (h % 2) * HD
            kTh = kT_sb[po : po + HD, it, :]
            qTh = qT_sb[po : po + HD, it, :]
            psctx = psc.tile([HD + 1, 1024], f32, name=f"ctx{h}", tag="ctx")
            ctx_tiles[h] = psctx
            for tt in range(NT):
                pss = psum_tile(f"pss{h}_{tt}")
                for c0 in (0, 512):
                    nc.tensor.matmul(
                        pss[:, c0 : c0 + 512],
                        lhsT=kTh[:, tt * P : (tt + 1) * P],
                        rhs=qTh[:, c0 : c0 + 512],
                        start=True,
                        stop=True,
                    )
                pt = ppool.tile([P, 1024], bf16, name="pt", tag="pt")
                nc.scalar.activation(
                    pt[:], pss[:], AF.Exp,
                    bias=mask_sb[:, tt : tt + 1], scale=SCALE,
                )
                for c0 in (0, 512):
                    nc.tensor.matmul(
                        psctx[:, c0 : c0 + 512],
                        lhsT=vaug_sb[:, tt, h * (HD + 1) : (h + 1) * (HD + 1)],
                        rhs=pt[:, c0 : c0 + 512],
                        start=(tt == 0),
                        stop=(tt == NT - 1),
                    )
            if h > 0:
                normalize(h - 1)
        normalize(H - 1)

        # ---- Phase 3: out proj + residual + LayerNorm ----
        for sc in range(NSH):
            pso = psum_tile(f"pso{sc}")
            for it in range(NJ):
                lhs = ctxT_sb[:, it, sc * P : (sc + 1) * P]
                nc.tensor.matmul(
                    pso[:, 0:512], lhsT=lhs, rhs=wo_sb[:, it, 0:512],
                    start=(it == 0), stop=(it == NJ - 1),
                )
                nc.tensor.matmul(
                    pso[:, 512:768], lhsT=lhs, rhs=wo_sb[:, it, 512:768],
                    start=(it == 0), stop=(it == NJ - 1),
                )
            xrt = xrpool.tile([P, DM], f32, name="xrt", tag="xr")
            nc.sync.dma_start(xrt[:], xr[sc * P : (sc + 1) * P, :])
            ht = hpool.tile([P, DM], f32, name="ht", tag="h")
            nc.vector.tensor_add(out=ht[:], in0=pso[:, 0:DM], in1=xrt[:])
            stats = stpool.tile([P, 3, 6], f32, name="stats", tag="st")
            for g in range(3):
                nc.vector.bn_stats(stats[:, g, :], ht[:, g * 256 : (g + 1) * 256])
            mv = stpool.tile([P, 2], f32, name="mv", tag="mv")
            nc.vector.bn_aggr(mv[:], stats[:])
            sq = stpool.tile([P, 1], f32, name="sq", tag="sq")
            nc.scalar.activation(sq[:], mv[:, 1:2], AF.Sqrt, bias=eps_sb[:])
            rstd = stpool.tile([P, 1], f32, name="rstd", tag="rstd")
            nc.vector.reciprocal(rstd[:], sq[:])
            ot = opool.tile([P, DM], f32, name="ot", tag="ot")
            nc.vector.tensor_scalar(
                out=ot[:], in0=ht[:],
                scalar1=mv[:, 0:1], scalar2=rstd[:],
                op0=ALU.subtract, op1=ALU.mult,
            )
            nc.sync.dma_start(out[sc * P : (sc + 1) * P, :], ot[:])

    nc.compile()
    return nc


_PROGRAM_CACHE: dict = {}


def _get_program(bq_nz, bk_nz, bv_nz):
    key = (bq_nz, bk_nz, bv_nz)
    if key not in _PROGRAM_CACHE:
        _PROGRAM_CACHE[key] = _build_program(*key)
    return _PROGRAM_CACHE[key]


def _prep_core_inputs(inputs, b, half):
    """Host-side shard prep for core (b, half). Keys are permuted so the core's
    own query half comes first; attention is permutation-invariant in t as long
    as k, v, mask and the additive tensors share the order."""
    x = np.asarray(inputs["hidden_states"][b], np.float32)          # [S, DM]
    if half == 0:
        t_order = slice(None)
        xh = x[:SH]
    else:
        t_order = np.r_[SH:S, 0:SH]
        xh = x[SH:]
    xp = x[t_order] if half else x                                  # [S, DM] permuted
    xT = np.ascontiguousarray(xp.T).astype(BF16)                    # [DM, S]
    xr = xh + np.asarray(inputs["bo"], np.float32)[None, :]         # residual + bo
    ak = np.asarray(inputs["addi_key"][b], np.float32)              # [NSYN, S, HD]
    ak = ak.transpose(0, 2, 1).reshape(NSYN * HD, S)
    av = np.asarray(inputs["addi_value"][b], np.float32)
    av = av.transpose(1, 0, 2).reshape(S, NSYN * HD)
    mask = np.asarray(inputs["attention_mask"][b, 0, 0], np.float32)
    if half:
        ak = ak[:, t_order]
        av = av[t_order]
        mask = mask[t_order]
    return {
        "xT": xT,
        "xr": np.ascontiguousarray(xr, np.float32),
        "addikT": np.ascontiguousarray(ak).astype(BF16),
        "addiv": np.ascontiguousarray(av).astype(BF16),
        "mask": np.ascontiguousarray(mask, np.float32),
    }


def _prep_in_maps(inputs):
    wqT = np.ascontiguousarray(np.asarray(inputs["Wq"], np.float32).T).astype(BF16)
    wkT = np.ascontiguousarray(np.asarray(inputs["Wk"], np.float32).T).astype(BF16)
    wvT = np.ascontiguousarray(np.asarray(inputs["Wv"], np.float32).T).astype(BF16)
    woT = np.ascontiguousarray(np.asarray(inputs["Wo"], np.float32).T).astype(BF16)
    shared = {
        "wqT": wqT, "wkT": wkT, "wvT": wvT, "woT": woT,
        "bq": np.asarray(inputs["bq"], np.float32),
        "bk": np.asarray(inputs["bk"], np.float32),
        "bv": np.asarray(inputs["bv"], np.float32),
    }
    in_maps = []
    for c in range(N_CORES):
        m = _prep_core_inputs(inputs, c // 2, c % 2)
        m.update(shared)
        in_maps.append(m)
    return in_maps


def _postprocess(inputs, results):
    out = np.empty((B, S, DM), np.float32)
    for c in range(N_CORES):
        b, half = c // 2, c % 2
        out[b, half * SH : (half + 1) * SH] = results[c]["out"]
    ln_g = np.asarray(inputs["ln_g"], np.float32)
    ln_b = np.asarray(inputs["ln_b"], np.float32)
    if np.any(ln_b) or not np.all(ln_g == 1.0):
        out = out * ln_g[None, None, :] + ln_b[None, None, :]
    return out


def run(inputs, trace=False, **kwargs):
    """Run on hardware; returns (full_output, BassKernelResults)."""
    nc = _get_program(
        bool(np.any(inputs["bq"])),
        bool(np.any(inputs["bk"])),
        bool(np.any(inputs["bv"])),
    )
    in_maps = _prep_in_maps(inputs)
    res = run_bass_kernel_spmd(
        nc, in_maps, core_ids=list(range(N_CORES)), trace=trace, **kwargs
    )
    return _postprocess(inputs, res.results), res


def kernel(**inputs) -> np.ndarray:
    out, _ = run(inputs)
    return out
